# revision 1
# baseline (speedup 1.0000x reference)
"""Trainium2 Bass kernel for fake-quant (W8A8) linear: y = fq_tok(x) @ fq_ch(w).T + b.

Full shapes: x [4, 2048, 4096] f32, w [4096, 4096] f32, b [4096] f32.
Sharding over 8 cores: 2 token groups x 4 out-channel groups.
Per core: x_sh [4096, 4096], w_sh [1024, 4096], b_sh [1024] -> y_sh [4096, 1024].

Key idea: quantized values are integers in [-127, 127], exactly representable
in bf16, so the matmul runs on the PE array in bf16 (full rate) with fp32 PSUM
accumulation - numerically equivalent to the fp32 reference einsum on the
dequantized values.  Scales are applied in the fp32 epilogue.

Rounding: round-half-to-even via the fp32 magic-constant trick
(v + 1.5*2^23 rounds mantissa to integer; subtract again afterwards),
matching jnp.round.  Clipping to [-128, 127] is a no-op by construction
(|x|/s <= 127 when s = amax/127) so it is skipped.

Engine split: DVE does amax + scale/reciprocal + the fp32 epilogue
(psum*sx*sw, +bias); ACT does the rounding multiply-add, the magic-subtract
(f32->bf16) and the PSUM->SBUF copies of PE-transposed tiles; PE does the
128x128 transposes (is_transpose matmul) + the bf16 matmuls.

Measured on 8 axon-tunneled TRN2 cores: ~649 us exec (max over cores),
relative error ~5e-5 vs the fp32 jax reference.  The main loop is a one-tile
software pipeline (tile tt's transposes emitted before tile tt-1's matmuls)
so the PSUM->SBUF copy tail hides under a matmul block; this needs PSUM
slack (psum 4 + tpsum 3 = 7/8 banks) - at 8/8 banks it faults the device.
"""

from contextlib import ExitStack

import numpy as np

import concourse.bass as bass
import concourse.mybir as mybir
import concourse.tile as tile
from concourse import bacc
from concourse.masks import make_identity

P = 128
MAGIC = 12582912.0  # 1.5 * 2**23
QMAX = 127.0
EPS = 1e-8

# full problem shapes (hardcoded per harness contract)
B, S, D_IN, D_OUT = 4, 2048, 4096, 4096
TOK = B * S  # 8192
TOK_GROUPS = 2
CH_GROUPS = 4
T_SH = TOK // TOK_GROUPS  # 4096 tokens per core
O_SH = D_OUT // CH_GROUPS  # 1024 channels per core


def build_nc(T, K, O, nch=512):
    """Build the per-core Bass program: x[T,K], w[O,K], b[O] -> y[T,O]."""
    f32 = mybir.dt.float32
    bf16 = mybir.dt.bfloat16
    Copy = mybir.ActivationFunctionType.Copy
    Alu = mybir.AluOpType
    AxX = mybir.AxisListType.X

    assert T % P == 0 and K % P == 0 and O % P == 0
    TT, KB, WT = T // P, K // P, O // P
    NCH = min(nch, O)
    CB = O // NCH

    nc = bacc.Bacc("TRN2", target_bir_lowering=False, debug=False)
    x_ap = nc.dram_tensor("x", [T, K], f32, kind="ExternalInput").ap()
    w_ap = nc.dram_tensor("w", [O, K], f32, kind="ExternalInput").ap()
    b_ap = nc.dram_tensor("b", [O], f32, kind="ExternalInput").ap()
    y_ap = nc.dram_tensor("y", [T, O], f32, kind="ExternalOutput").ap()

    with tile.TileContext(nc) as tc, ExitStack() as ctx:
        singles = ctx.enter_context(tc.tile_pool(name="singles", bufs=1))
        bigf32 = ctx.enter_context(tc.tile_pool(name="bigf32", bufs=3))
        rnd = ctx.enter_context(tc.tile_pool(name="rnd", bufs=2))
        qpool = ctx.enter_context(tc.tile_pool(name="qpool", bufs=2))
        qtpool = ctx.enter_context(tc.tile_pool(name="qtpool", bufs=3))
        stats = ctx.enter_context(tc.tile_pool(name="stats", bufs=8))
        opool = ctx.enter_context(tc.tile_pool(name="opool", bufs=4))
        psum_pool = ctx.enter_context(tc.tile_pool(name="psum", bufs=4, space="PSUM"))
        tpsum = ctx.enter_context(tc.tile_pool(name="tpsum", bufs=3, space="PSUM"))
        dram = ctx.enter_context(tc.tile_pool(name="dram", bufs=1, space="DRAM"))

        # resident: transposed quantized weights + broadcast scale/bias rows
        qwT = singles.tile([P, KB, O], bf16)  # qwT[f, k, c] = qw[c, k*128+f]
        sw_b = singles.tile([P, O], f32)
        bb_b = singles.tile([P, O], f32)
        sw_dram = dram.tile([O, 1], f32)
        ident = singles.tile([P, P], bf16)
        make_identity(nc, ident)

        TG = min(8, KB)  # k-blocks per PE-transpose psum group (8*128 bf16 = one bank)

        def pe_transpose(q_sbuf, dst, tag):
            # q_sbuf [P, K] bf16 -> dst [P, KB, P] slice view with
            # dst[f, k, c] = q_sbuf[c, k*128+f], via PE transpose + ACT copy
            for g in range(KB // TG):
                tp = tpsum.tile([P, TG, P], bf16, tag="tp", name=f"tp_{tag}_{g}")
                for j in range(TG):
                    kb = g * TG + j
                    nc.tensor.transpose(
                        tp[:, j, :], q_sbuf[:, kb * P : (kb + 1) * P], ident
                    )
                nc.scalar.activation(
                    out=dst[:, g * TG : (g + 1) * TG, :], in_=tp, func=Copy
                )

        def quantize(src_t, q_t, s_t, dve_round=False):
            # per-row amax -> scale (s_t), then round src*(1/s) to q_t (bf16)
            amax = stats.tile([P, 1], f32, tag="st", name="amax")
            nc.vector.reduce_max(
                out=amax, in_=src_t, axis=AxX, apply_absolute_value=True
            )
            nc.vector.tensor_scalar(
                out=s_t, in0=amax, scalar1=1.0 / QMAX, scalar2=EPS,
                op0=Alu.mult, op1=Alu.max,
            )
            r_t = stats.tile([P, 1], f32, tag="st", name="recip")
            nc.vector.reciprocal(out=r_t, in_=s_t)
            t_t = rnd.tile([P, K], f32, tag="rnd", name="t_round")
            # round on ACT (scale is a per-partition pointer operand; the
            # Bacc event-semaphore pass legalizes its single-wait limit)
            if dve_round:
                # weight phase: DVE does the round so ACT (busy with copies
                # and x rounds during the ramp) is off the critical path
                nc.vector.tensor_scalar(
                    out=t_t, in0=src_t, scalar1=r_t[:, 0:1], scalar2=MAGIC,
                    op0=Alu.mult, op1=Alu.add,
                )
            else:
                nc.scalar.activation(
                    out=t_t, in_=src_t, func=Copy, bias=MAGIC, scale=r_t[:, 0:1]
                )
            nc.scalar.activation(out=q_t, in_=t_t, func=Copy, bias=-MAGIC, scale=1.0)

        # ---- weight phase: quantize w per-channel, transpose to [K, O] ----
        for wt in range(WT):
            w_t = bigf32.tile([P, K], f32, tag="big", name=f"w_{wt}")
            nc.sync.dma_start(out=w_t, in_=w_ap[wt * P : (wt + 1) * P, :])
            sw = stats.tile([P, 1], f32, tag="st", name=f"sw_{wt}")
            qw = qpool.tile([P, K], bf16, tag="q", name=f"qw_{wt}")
            quantize(w_t, qw, sw, dve_round=True)
            pe_transpose(qw, qwT[:, :, wt * P : (wt + 1) * P], f"w{wt}")
            nc.sync.dma_start(out=sw_dram[wt * P : (wt + 1) * P, :], in_=sw)

        # broadcast per-channel scale & bias across partitions
        nc.sync.dma_start(
            out=sw_b,
            in_=bass.AP(tensor=sw_dram.tensor, offset=sw_dram.offset, ap=[[0, P], [1, O]]),
        )
        nc.sync.dma_start(
            out=bb_b,
            in_=bass.AP(tensor=b_ap.tensor, offset=b_ap.offset, ap=[[0, P], [1, O]]),
        )

        # ---- main loop over token tiles (one-tile software pipeline:
        # tile tt's transposes are emitted BEFORE tile tt-1's matmuls so
        # the PSUM->SBUF copy tail hides under a full matmul block) ----
        def matmul_block(tt, sx, qxT):
            psums = [
                psum_pool.tile([P, NCH], f32, tag="psum", name=f"psum_{tt}_{cb}")
                for cb in range(CB)
            ]
            for k in range(KB):
                for cb in range(CB):
                    nc.tensor.matmul(
                        psums[cb],
                        qxT[:, k, :],
                        qwT[:, k, cb * NCH : (cb + 1) * NCH],
                        start=(k == 0),
                        stop=(k == KB - 1),
                    )
            for cb in range(CB):
                o1 = opool.tile([P, NCH], f32, tag="o", name=f"o1_{tt}_{cb}")
                nc.vector.scalar_tensor_tensor(
                    out=o1, in0=psums[cb], scalar=sx[:, 0:1],
                    in1=sw_b[:, cb * NCH : (cb + 1) * NCH],
                    op0=Alu.mult, op1=Alu.mult,
                )
                o2 = opool.tile([P, NCH], f32, tag="o", name=f"o2_{tt}_{cb}")
                nc.vector.tensor_add(
                    out=o2, in0=o1, in1=bb_b[:, cb * NCH : (cb + 1) * NCH]
                )
                nc.sync.dma_start(
                    out=y_ap[tt * P : (tt + 1) * P, cb * NCH : (cb + 1) * NCH], in_=o2
                )

        pending = None
        for tt in range(TT):
            x_t = bigf32.tile([P, K], f32, tag="big", name=f"x_{tt}")
            nc.sync.dma_start(out=x_t, in_=x_ap[tt * P : (tt + 1) * P, :])
            sx = stats.tile([P, 1], f32, tag="st", name=f"sx_{tt}")
            qx = qpool.tile([P, K], bf16, tag="q", name=f"qx_{tt}")
            quantize(x_t, qx, sx)
            qxT = qtpool.tile([P, KB, P], bf16)  # qxT[f, k, t] = qx[t, k*128+f]
            pe_transpose(qx, qxT, f"x{tt}")
            if pending is not None:
                matmul_block(*pending)
            pending = (tt, sx, qxT)
        matmul_block(*pending)
    nc.compile()
    return nc


_cached_nc = None


def _get_nc():
    global _cached_nc
    if _cached_nc is None:
        _cached_nc = build_nc(T_SH, D_IN, O_SH)
    return _cached_nc


def kernel(x: np.ndarray, w: np.ndarray, b: np.ndarray, _trace=False):
    from concourse.bass_utils import run_bass_kernel_spmd

    assert x.shape == (B, S, D_IN) and w.shape == (D_OUT, D_IN) and b.shape == (D_OUT,)
    x2 = np.ascontiguousarray(x.reshape(TOK, D_IN), dtype=np.float32)
    w2 = np.ascontiguousarray(w, dtype=np.float32)
    b2 = np.ascontiguousarray(b, dtype=np.float32)

    in_maps = []
    for core in range(8):
        tg, cg = divmod(core, CH_GROUPS)
        in_maps.append(
            {
                "x": np.ascontiguousarray(x2[tg * T_SH : (tg + 1) * T_SH]),
                "w": np.ascontiguousarray(w2[cg * O_SH : (cg + 1) * O_SH]),
                "b": np.ascontiguousarray(b2[cg * O_SH : (cg + 1) * O_SH]),
            }
        )

    nc = _get_nc()
    res = run_bass_kernel_spmd(nc, in_maps, core_ids=list(range(8)), trace=_trace)

    y = np.empty((TOK, D_OUT), dtype=np.float32)
    for core in range(8):
        tg, cg = divmod(core, CH_GROUPS)
        y[tg * T_SH : (tg + 1) * T_SH, cg * O_SH : (cg + 1) * O_SH] = res.results[
            core
        ]["y"]
    if _trace:
        kernel._last_results = res
    return y.reshape(B, S, D_OUT)



# revision 3
# speedup vs baseline: 1.1825x; 1.1825x over previous
"""Trainium2 Bass kernel for fake-quant (W8A8) linear: y = fq_tok(x) @ fq_ch(w).T + b.

Full shapes: x [4, 2048, 4096] f32, w [4096, 4096] f32, b [4096] f32.
Sharding over 8 cores: 2 token groups x 4 out-channel groups.
Per core: x_sh [4096, 4096], w_sh [1024, 4096], b_sh [1024] -> y_sh [4096, 1024].

Key idea: quantized values are integers in [-127, 127], exactly representable
in bf16, so the matmul runs on the PE array in bf16 (full rate) with fp32 PSUM
accumulation - numerically equivalent to the fp32 reference einsum on the
dequantized values.  Scales are applied in the fp32 epilogue.

Rounding: round-half-to-even via the fp32 magic-constant trick
(v + 1.5*2^23 rounds mantissa to integer; subtract again afterwards),
matching jnp.round.  Clipping to [-128, 127] is a no-op by construction
(|x|/s <= 127 when s = amax/127) so it is skipped.

v2 changes over the 753us baseline (all aimed at PE saturation - measured
PE steady state is gap-free; all idle was in the first ~150us):
 - qwT is split into two channel-half tiles so tile-0 matmuls for channels
   [0,512) only wait on w-tiles 0-3, not all 8.
 - head emission order interleaves x-tile prefetch with the weight phase
   (w0-3, x0, w4-7, x1, ...), so ACT/DVE fill the PE with x transposes and
   early matmuls while the rest of W streams in.
 - per-tile matmuls grouped per channel half (cb), epilogue per half right
   after its 32-matmul accumulation -> tighter psum rotation.
 - the 4 per-tile PSUM->SBUF transpose-drain copies are split 2/2 between
   ACT and DVE (ACT was the secondary bottleneck at 68% busy).

Engine budget per x tile (measured rates): PE 17.0us (64 matmuls @233.6ns +
32 transposes @64.8ns), ACT ~11.7us (round + magic-sub + 2 copies),
DVE ~11.7us (amax + recip + epilogue + 2 copies).
"""

from contextlib import ExitStack

import numpy as np

import concourse.bass as bass
import concourse.mybir as mybir
import concourse.tile as tile
from concourse import bacc
from concourse.masks import make_identity

P = 128
MAGIC = 12582912.0  # 1.5 * 2**23
QMAX = 127.0
EPS = 1e-8

# full problem shapes (hardcoded per harness contract)
B, S, D_IN, D_OUT = 4, 2048, 4096, 4096
TOK = B * S  # 8192
TOK_GROUPS = 2
CH_GROUPS = 4
T_SH = TOK // TOK_GROUPS  # 4096 tokens per core
O_SH = D_OUT // CH_GROUPS  # 1024 channels per core


def build_nc(T, K, O, nch=512):
    """Build the per-core Bass program: x[T,K], w[O,K], b[O] -> y[T,O]."""
    f32 = mybir.dt.float32
    bf16 = mybir.dt.bfloat16
    Copy = mybir.ActivationFunctionType.Copy
    Alu = mybir.AluOpType
    AxX = mybir.AxisListType.X

    assert T % P == 0 and K % P == 0 and O % P == 0
    TT, KB, WT = T // P, K // P, O // P
    NCH = min(nch, O)
    CB = O // NCH  # channel halves (2)
    WPH = WT // CB  # w tiles per channel half (4)

    nc = bacc.Bacc("TRN2", target_bir_lowering=False, debug=False)
    x_ap = nc.dram_tensor("x", [T, K], f32, kind="ExternalInput").ap()
    w_ap = nc.dram_tensor("w", [O, K], f32, kind="ExternalInput").ap()
    b_ap = nc.dram_tensor("b", [O], f32, kind="ExternalInput").ap()
    y_ap = nc.dram_tensor("y", [T, O], f32, kind="ExternalOutput").ap()

    with tile.TileContext(nc) as tc, ExitStack() as ctx:
        singles = ctx.enter_context(tc.tile_pool(name="singles", bufs=1))
        bigf32 = ctx.enter_context(tc.tile_pool(name="bigf32", bufs=3))
        rnd = ctx.enter_context(tc.tile_pool(name="rnd", bufs=2))
        qpool = ctx.enter_context(tc.tile_pool(name="qpool", bufs=2))
        qtpool = ctx.enter_context(tc.tile_pool(name="qtpool", bufs=3))
        stats = ctx.enter_context(tc.tile_pool(name="stats", bufs=8))
        opool = ctx.enter_context(tc.tile_pool(name="opool", bufs=4))
        psum_pool = ctx.enter_context(tc.tile_pool(name="psum", bufs=4, space="PSUM"))
        tpsum = ctx.enter_context(tc.tile_pool(name="tpsum", bufs=3, space="PSUM"))
        dram = ctx.enter_context(tc.tile_pool(name="dram", bufs=1, space="DRAM"))

        # resident: transposed quantized weights (split in two channel
        # halves so early matmuls only depend on half the weight phase)
        # qwT_h[cb][f, k, c] = qw[cb*NCH + c, k*128+f]
        qwT_h = [
            singles.tile([P, KB, NCH], bf16, name=f"qwT_h{i}") for i in range(CB)
        ]
        sw_b = singles.tile([P, O], f32)
        bb_b = singles.tile([P, O], f32)
        sw_dram = dram.tile([O, 1], f32)
        ident = singles.tile([P, P], bf16)
        make_identity(nc, ident)

        TG = min(8, KB)  # k-blocks per PE-transpose psum group (8*128 bf16 = one bank)

        def pe_transpose(q_sbuf, dst, tag, dst_col_base=0):
            # q_sbuf [P, K] bf16 -> dst [P, KB, *] slice view with
            # dst[f, k, dst_col_base + c] = q_sbuf[c, k*128+f]
            # PE transposes into PSUM; drain copies split between ACT (even
            # groups) and DVE (odd groups) to balance the two engines.
            for g in range(KB // TG):
                tp = tpsum.tile([P, TG, P], bf16, tag="tp", name=f"tp_{tag}_{g}")
                for j in range(TG):
                    kb = g * TG + j
                    nc.tensor.transpose(
                        tp[:, j, :], q_sbuf[:, kb * P : (kb + 1) * P], ident
                    )
                dst_sl = dst[:, g * TG : (g + 1) * TG,
                             dst_col_base : dst_col_base + P]
                if g % 2 == 0:
                    nc.scalar.activation(out=dst_sl, in_=tp, func=Copy)
                else:
                    nc.vector.tensor_copy(dst_sl, tp)

        def quantize(src_t, q_t, s_t, dve_round=False):
            # per-row amax -> scale (s_t), then round src*(1/s) to q_t (bf16)
            amax = stats.tile([P, 1], f32, tag="st", name="amax")
            nc.vector.reduce_max(
                out=amax, in_=src_t, axis=AxX, apply_absolute_value=True
            )
            nc.vector.tensor_scalar(
                out=s_t, in0=amax, scalar1=1.0 / QMAX, scalar2=EPS,
                op0=Alu.mult, op1=Alu.max,
            )
            r_t = stats.tile([P, 1], f32, tag="st", name="recip")
            nc.vector.reciprocal(out=r_t, in_=s_t)
            t_t = rnd.tile([P, K], f32, tag="rnd", name="t_round")
            # round on ACT (scale is a per-partition pointer operand; the
            # Bacc event-semaphore pass legalizes its single-wait limit)
            if dve_round:
                # weight phase: DVE does the round so ACT (busy with copies
                # and x rounds during the ramp) is off the critical path
                nc.vector.tensor_scalar(
                    out=t_t, in0=src_t, scalar1=r_t[:, 0:1], scalar2=MAGIC,
                    op0=Alu.mult, op1=Alu.add,
                )
            else:
                nc.scalar.activation(
                    out=t_t, in_=src_t, func=Copy, bias=MAGIC, scale=r_t[:, 0:1]
                )
            nc.scalar.activation(out=q_t, in_=t_t, func=Copy, bias=-MAGIC, scale=1.0)

        def process_w_tile(wt):
            w_t = bigf32.tile([P, K], f32, tag="big", name=f"w_{wt}")
            nc.sync.dma_start(out=w_t, in_=w_ap[wt * P : (wt + 1) * P, :])
            sw = stats.tile([P, 1], f32, tag="st", name=f"sw_{wt}")
            qw = qpool.tile([P, K], bf16, tag="q", name=f"qw_{wt}")
            quantize(w_t, qw, sw, dve_round=True)
            cb, sub = divmod(wt, WPH)
            pe_transpose(qw, qwT_h[cb], f"w{wt}", dst_col_base=sub * P)
            nc.sync.dma_start(out=sw_dram[wt * P : (wt + 1) * P, :], in_=sw)

        def load_quant_transpose_x(tt):
            x_t = bigf32.tile([P, K], f32, tag="big", name=f"x_{tt}")
            nc.sync.dma_start(out=x_t, in_=x_ap[tt * P : (tt + 1) * P, :])
            sx = stats.tile([P, 1], f32, tag="st", name=f"sx_{tt}")
            qx = qpool.tile([P, K], bf16, tag="q", name=f"qx_{tt}")
            quantize(x_t, qx, sx)
            qxT = qtpool.tile([P, KB, P], bf16)  # qxT[f, k, t] = qx[t, k*128+f]
            pe_transpose(qx, qxT, f"x{tt}")
            return sx, qxT

        def matmul_block(tt, sx, qxT):
            # per channel half: 32-matmul accumulation, then its epilogue
            for cb in range(CB):
                psum = psum_pool.tile(
                    [P, NCH], f32, tag="psum", name=f"psum_{tt}_{cb}"
                )
                for k in range(KB):
                    nc.tensor.matmul(
                        psum,
                        qxT[:, k, :],
                        qwT_h[cb][:, k, :],
                        start=(k == 0),
                        stop=(k == KB - 1),
                    )
                o1 = opool.tile([P, NCH], f32, tag="o", name=f"o1_{tt}_{cb}")
                nc.vector.scalar_tensor_tensor(
                    out=o1, in0=psum, scalar=sx[:, 0:1],
                    in1=sw_b[:, cb * NCH : (cb + 1) * NCH],
                    op0=Alu.mult, op1=Alu.mult,
                )
                o2 = opool.tile([P, NCH], f32, tag="o", name=f"o2_{tt}_{cb}")
                nc.vector.tensor_add(
                    out=o2, in0=o1, in1=bb_b[:, cb * NCH : (cb + 1) * NCH]
                )
                nc.sync.dma_start(
                    out=y_ap[tt * P : (tt + 1) * P, cb * NCH : (cb + 1) * NCH],
                    in_=o2,
                )

        # ---- head: weight phase interleaved with x-tile prefetch ----
        # w tiles 0-3 fill qwT_h[0] (channels [0,512)); then x0 can matmul
        # against half the channels while w4-7 are still quantizing.
        for wt in range(WPH):
            process_w_tile(wt)
        pending = (0, *load_quant_transpose_x(0))
        for wt in range(WPH, WT):
            process_w_tile(wt)

        # broadcast per-channel scale & bias across partitions
        nc.sync.dma_start(
            out=sw_b,
            in_=bass.AP(tensor=sw_dram.tensor, offset=sw_dram.offset, ap=[[0, P], [1, O]]),
        )
        nc.sync.dma_start(
            out=bb_b,
            in_=bass.AP(tensor=b_ap.tensor, offset=b_ap.offset, ap=[[0, P], [1, O]]),
        )

        # ---- main loop (one-tile software pipeline: tile tt's transposes
        # are emitted BEFORE tile tt-1's matmuls so the PSUM->SBUF copy tail
        # hides under a full matmul block) ----
        for tt in range(1, TT):
            nxt = (tt, *load_quant_transpose_x(tt))
            matmul_block(*pending)
            pending = nxt
        matmul_block(*pending)
    nc.compile()
    return nc


_cached_nc = None


def _get_nc():
    global _cached_nc
    if _cached_nc is None:
        _cached_nc = build_nc(T_SH, D_IN, O_SH)
    return _cached_nc


def kernel(x: np.ndarray, w: np.ndarray, b: np.ndarray, _trace=False):
    from concourse.bass_utils import run_bass_kernel_spmd

    assert x.shape == (B, S, D_IN) and w.shape == (D_OUT, D_IN) and b.shape == (D_OUT,)
    x2 = np.ascontiguousarray(x.reshape(TOK, D_IN), dtype=np.float32)
    w2 = np.ascontiguousarray(w, dtype=np.float32)
    b2 = np.ascontiguousarray(b, dtype=np.float32)

    in_maps = []
    for core in range(8):
        tg, cg = divmod(core, CH_GROUPS)
        in_maps.append(
            {
                "x": np.ascontiguousarray(x2[tg * T_SH : (tg + 1) * T_SH]),
                "w": np.ascontiguousarray(w2[cg * O_SH : (cg + 1) * O_SH]),
                "b": np.ascontiguousarray(b2[cg * O_SH : (cg + 1) * O_SH]),
            }
        )

    nc = _get_nc()
    res = run_bass_kernel_spmd(nc, in_maps, core_ids=list(range(8)), trace=_trace)

    y = np.empty((TOK, D_OUT), dtype=np.float32)
    for core in range(8):
        tg, cg = divmod(core, CH_GROUPS)
        y[tg * T_SH : (tg + 1) * T_SH, cg * O_SH : (cg + 1) * O_SH] = res.results[
            core
        ]["y"]
    if _trace:
        kernel._last_results = res
    return y.reshape(B, S, D_OUT)
